# revision 12
# baseline (speedup 1.0000x reference)
"""DGAD net (vq_codebook) kernel for 8x Trainium2 NeuronCores.

Contract: kernel(**inputs) takes the FULL unsharded inputs (numpy, keyed as in
setup_inputs) and returns the FULL [4,1] float32 output. Batch (128) is
sharded 16-per-core across 8 cores (data parallel); weights replicated. Each
core emits [1,4] partial sums (ce, origin_svdd, class_svdd, align); the final
all-reduce (sum across 8 cores, /128) happens on host during unshard.

Key moves vs the streaming cost:
  - mean_HW(conv1x1(x_mid, w)) == w @ mean_HW(x_mid): the conv collapses to
    pooling x_mid (25.7MB/core of streaming) plus a tiny matmul.
  - w_shallow and sw1 are both linear before the first leaky_relu, so they are
    FOLDED on host into one [512->1024] fp8 weight: -2.5MB of DMA and no
    in-stream conv matmuls at all.
  - concat([x, x - c]) @ W.T folds to x @ (Wa+Wb).T - c @ Wb.T: tw1/cw1 become
    64-wide weights plus a bias (tw1: per-partition ACT bias; cw1: a [4,64]
    bias matrix contracted with the one-hot on the PE).
  - All fp8 weights and all f32 consts are host-packed per-partition
    contiguous (128 descriptors per DMA instead of 2048 1KB ones).
  - The first shallow layer accumulates INCREMENTALLY in PSUM as each x_mid
    channel-chunk is pooled (matmul is linear), and the last chunk is split
    small, so the post-stream serial tail is short.
  - Prelu (== leaky_relu via alpha) lives in the same ACT table set as
    Exp/Ln/Square/Copy/Abs, so there is no mid-kernel table switch; svdd uses
    Square(bias=-center) to fuse subtract+square.
"""

import numpy as np
import ml_dtypes

N_CORES = 8
B = 128
BC = B // N_CORES  # 16 samples per core

BF = ml_dtypes.bfloat16
F8 = ml_dtypes.float8_e4m3
WSCALE = 256.0  # fp8 weights stored *256; 1/256 folded into consumer scales

# const blob column offsets (f32 blob is [128, 512])
C_TW1, C_TW2, C_CW1, C_CW2, C_QW1, C_QW2 = 0, 64, 128, 192, 256, 320
C_RSIM, C_CBIAS, C_TBIAS, C_CEN, C_NCEN = 384, 388, 452, 453, 454
C_PNORM, C_ID16, C_ONES, C_ONEROW = 455, 459, 475, 476
C_K = 512

_CACHE = {}


def _build_program():
    import concourse.bass as bass  # noqa: F401
    import concourse.mybir as mybir
    import concourse.tile as tile
    from concourse import bacc
    from contextlib import ExitStack

    dt = mybir.dt
    AF = mybir.ActivationFunctionType
    ALU = mybir.AluOpType
    AX = mybir.AxisListType
    f32, bf16, f8 = dt.float32, dt.bfloat16, dt.float8e4
    INV = 1.0 / WSCALE

    from concourse.hw_specs import get_activation_tables
    _act_set_id = list(get_activation_tables("gen3")).index(
        "natural_log_exp_and_others")

    nc = bacc.Bacc("TRN2", target_bir_lowering=False, debug=False,
                   enable_asserts=True, num_devices=N_CORES)

    def din(name, shape, d):
        return nc.dram_tensor(name, shape, d, kind="ExternalInput").ap()

    xm = din("xm", [BC, 512, 784], f32)
    xd = din("xd", [BC, 100352], f32)
    ow1_d = din("ow1T", [128, 16, 1024], f8)  # [p, j, o], d = 16p + j
    wf_d = din("wfT", [128, 4, 1024], f8)     # folded sw1@wsh, c = 128*cc + p
    ow2_d = din("ow2T", [128, 8, 512], f8)    # f = 128k + p
    sw2_d = din("sw2T", [128, 8, 512], f8)
    ow3_d = din("ow3T", [128, 4, 64], f8)
    sw3_d = din("sw3T", [128, 4, 64], f8)
    cst_d = din("cst", [128, C_K], f32)
    out_d = nc.dram_tensor("out", [1, 4], f32, kind="ExternalOutput").ap()

    with tile.TileContext(nc) as tc, ExitStack() as ctx:
        wp = ctx.enter_context(tc.tile_pool(name="wp", bufs=1))
        xp = ctx.enter_context(tc.tile_pool(name="xp", bufs=5))
        ap_ = ctx.enter_context(tc.tile_pool(name="ap", bufs=1))
        pp = ctx.enter_context(tc.tile_pool(name="pp", bufs=2, space="PSUM"))
        pt = ctx.enter_context(tc.tile_pool(name="pt", bufs=4, space="PSUM"))
        ps1 = ctx.enter_context(tc.tile_pool(name="ps1", bufs=1, space="PSUM"))
        ps2 = ctx.enter_context(tc.tile_pool(name="ps2", bufs=1, space="PSUM"))

        # Preload the one ACT table set covering Prelu/Exp/Ln/Square/Copy/Abs
        # so the auto-pass never inserts a mid-kernel table switch.
        ldset = mybir.InstLoadActFuncSet(
            name=f"I-{nc.next_id()}", act_func_set_id=_act_set_id,
            ins=[], outs=[])
        ldset.engine = mybir.EngineType.Activation
        nc.scalar.add_instruction(ldset)

        # ---------------- DMA issue order ----------------
        ow1_t = wp.tile([128, 16, 1024], f8, tag="ow1")
        wf_t = wp.tile([128, 4, 1024], f8, tag="wf")
        ow2_t = wp.tile([128, 8, 512], f8, tag="ow2")
        sw2_t = wp.tile([128, 8, 512], f8, tag="sw2")
        ow3_t = wp.tile([128, 4, 64], f8, tag="ow3")
        sw3_t = wp.tile([128, 4, 64], f8, tag="sw3")
        cst = wp.tile([128, C_K], f32, tag="cst")
        for t_, d_ in ((ow1_t, ow1_d), (wf_t, wf_d), (ow2_t, ow2_d),
                       (sw2_t, sw2_d), (ow3_t, ow3_d), (sw3_t, sw3_d),
                       (cst, cst_d)):
            nc.sync.dma_start(out=t_[:], in_=d_)

        # x_deep: one resident tile, 4 slice DMAs (no buffer waits on Sync)
        xd_t = ap_.tile([128, 16, 16, 49], f32, tag="xd")
        for g in range(4):
            nc.sync.dma_start(
                out=xd_t[:, 4 * g:4 * g + 4],
                in_=xd[4 * g:4 * g + 4].rearrange("b (p j h) -> p b j h",
                                                  p=128, j=16, h=49))

        # x_mid: 12 chunks of 4 samples + last cc split finely so the DVE
        # reduces keep up with (and finish right after) the last DMA bytes.
        # Non-4 sizes get their own single-buffer tags so consecutive small
        # chunks never serialize DMA behind a reduce.
        XM_SUB = [(0, 4), (4, 4), (8, 4), (12, 4)]
        XM_SUB_LAST = [(0, 4), (4, 4), (8, 2), (10, 2), (12, 2), (14, 1), (15, 1)]
        xm_tiles = {}
        small_i = 0
        for cc in range(4):
            subs = XM_SUB if cc < 3 else XM_SUB_LAST
            for (b0, nb) in subs:
                if nb == 4:
                    t = xp.tile([128, 4, 784], f32, tag="xm4")
                else:
                    t = xp.tile([128, nb, 784], f32, tag=f"xms{small_i}", bufs=1)
                    small_i += 1
                nc.sync.dma_start(
                    out=t[:],
                    in_=xm[b0:b0 + nb, cc * 128:(cc + 1) * 128, :]
                    .rearrange("b c h -> c b h"))
                xm_tiles[(cc, b0)] = t

        # const blob views
        tw1_l = cst[0:64, C_TW1:C_TW1 + 64]
        tw2_l = cst[0:64, C_TW2:C_TW2 + 64]
        cw1_l = cst[0:64, C_CW1:C_CW1 + 64]
        cw2_l = cst[0:64, C_CW2:C_CW2 + 64]
        qw1_l = cst[0:64, C_QW1:C_QW1 + 64]
        qw2_l = cst[0:64, C_QW2:C_QW2 + 64]
        rsim = cst[0:64, C_RSIM:C_RSIM + 4]
        cbias = cst[0:4, C_CBIAS:C_CBIAS + 64]
        tbias = cst[0:64, C_TBIAS:C_TBIAS + 1]
        ncen = cst[0:64, C_NCEN:C_NCEN + 1]
        pnorm = cst[0:1, C_PNORM:C_PNORM + 4]
        id16 = cst[0:16, C_ID16:C_ID16 + 16]
        ones64 = cst[0:64, C_ONES:C_ONES + 1]
        ones16 = cst[0:16, C_ONES:C_ONES + 1]
        onerow = cst[0:1, C_ONEROW:C_ONEROW + 16]

        # ---------------- x_deep pool (DVE; runs before x_mid arrives) ------
        xdsum = ap_.tile([128, 16, 16], f32, tag="xdsum")
        for g in range(4):
            nc.vector.reduce_sum(xdsum[:, 4 * g:4 * g + 4, :],
                                 xd_t[:, 4 * g:4 * g + 4, :, :], axis=AX.X)
        xdb = ap_.tile([128, 16, 16], bf16, tag="xdb")  # [p, b, j]
        nc.vector.tensor_scalar(xdb[:], xdsum[:], INV / 49.0, None, op0=ALU.mult)

        # ---------------- origin chain (PE/ACT, during x_mid stream) --------
        y1o_ps = pp.tile([128, 128], f32, tag="mm")
        for m in range(8):
            for j in range(16):
                nc.tensor.matmul(y1o_ps[:, m * 16:(m + 1) * 16],
                                 ow1_t[:, j, m * 128:(m + 1) * 128],
                                 xdb[:, :, j], start=(j == 0), stop=(j == 15))
        y1o = ap_.tile([128, 128], bf16, tag="y1o")
        nc.scalar.activation(y1o[:], y1o_ps[:], AF.Prelu, alpha=0.01)

        y2o_ps = pp.tile([128, 64], f32, tag="mm")
        for m in range(4):
            for k in range(8):
                nc.tensor.matmul(y2o_ps[:, m * 16:(m + 1) * 16],
                                 ow2_t[:, k, m * 128:(m + 1) * 128],
                                 y1o[:, k * 16:(k + 1) * 16],
                                 start=(k == 0), stop=(k == 7))
        y2o = ap_.tile([128, 64], bf16, tag="y2o")
        nc.scalar.activation(y2o[:], y2o_ps[:], AF.Prelu, scale=INV, alpha=0.01)

        orig_ps = pt.tile([128, 16], f32, tag="tail")
        for k in range(4):
            nc.tensor.matmul(orig_ps[:64, :], ow3_t[:, k, :],
                             y2o[:, k * 16:(k + 1) * 16],
                             start=(k == 0), stop=(k == 3))
        origin = ap_.tile([64, 16], f32, tag="origin")
        nc.scalar.activation(origin[:], orig_ps[:64, :], AF.Prelu,
                             scale=INV, alpha=0.01)

        # qw chain + origin_svdd (all during stream)
        q1_ps = pt.tile([128, 16], f32, tag="tail")
        nc.tensor.matmul(q1_ps[:64, :], qw1_l, origin[:], start=True, stop=True)
        q1 = ap_.tile([64, 16], f32, tag="q1")
        nc.scalar.activation(q1[:], q1_ps[:64, :], AF.Prelu, alpha=0.01)
        q2_ps = pt.tile([128, 16], f32, tag="tail")
        nc.tensor.matmul(q2_ps[:64, :], qw2_l, q1[:], start=True, stop=True)
        qf = ap_.tile([64, 16], f32, tag="qf")
        nc.scalar.activation(qf[:], q2_ps[:64, :], AF.Prelu, alpha=0.01)
        qsq = ap_.tile([64, 16], f32, tag="qsq")
        nc.scalar.activation(qsq[:], qf[:], AF.Square, bias=ncen)

        # class-chain first matmul: start accumulation now, finish post-onehot
        cat2_ps = ps2.tile([64, 16], f32, tag="cat2")
        nc.tensor.matmul(cat2_ps[:], cw1_l, origin[:], start=True, stop=False)

        osv_ps = pt.tile([128, 16], f32, tag="tail")
        nc.tensor.matmul(osv_ps[0:1, :], ones64, qsq[:], start=True, stop=True)
        osvdd = ap_.tile([1, 16], f32, tag="osvdd")
        nc.scalar.copy(osvdd[:], osv_ps[0:1, :])

        # ---------------- x_mid stream: pool + incremental folded layer -----
        # Pooling is split DVE/ACT per chunk: DVE reduce_sum takes the first
        # half of each chunk's samples, ACT Copy+accum_out (a per-partition
        # free-axis sum) takes the rest, halving DVE load (DVE alone is only
        # 15% faster than the DMA delivery rate).
        xmsum = ap_.tile([128, 4, 16], f32, tag="xmsum")
        xmb = ap_.tile([128, 4, 16], bf16, tag="xmb")
        act_scr = ap_.tile([128, 784], f32, tag="act_scr")
        y1s_ps = ps1.tile([128, 128], f32, tag="y1s")

        def y1s_mm(cc, b0, nb, start, stop):
            for m in range(8):
                nc.tensor.matmul(y1s_ps[:, m * 16 + b0:m * 16 + b0 + nb],
                                 wf_t[:, cc, m * 128:(m + 1) * 128],
                                 xmb[:, cc, b0:b0 + nb], start=start, stop=stop)

        def pool_chunk(cc, b0, nb, t):
            nd = (nb + 1) // 2  # DVE samples; ACT takes the rest
            if nd:
                nc.vector.reduce_sum(xmsum[:, cc, b0:b0 + nd],
                                     t[:, 0:nd, :], axis=AX.X)
            for i in range(nd, nb):
                nc.scalar.activation(act_scr[:], t[:, i, :], AF.Copy,
                                     accum_out=xmsum[:, cc, b0 + i:b0 + i + 1])

        for cc in range(4):
            subs = XM_SUB if cc < 3 else XM_SUB_LAST
            for si, (b0, nb) in enumerate(subs):
                t = xm_tiles[(cc, b0)]
                if cc == 3 and nb == 1:
                    # run the two final 1-sample pools on different engines
                    if si == len(subs) - 2:
                        nc.vector.reduce_sum(xmsum[:, cc, b0:b0 + 1],
                                             t[:, 0:1, :], axis=AX.X)
                    else:
                        nc.scalar.activation(
                            act_scr[:], t[:, 0, :], AF.Copy,
                            accum_out=xmsum[:, cc, b0:b0 + 1])
                else:
                    pool_chunk(cc, b0, nb, t)
                if cc == 3:
                    nc.vector.tensor_scalar(xmb[:, cc, b0:b0 + nb],
                                            xmsum[:, cc, b0:b0 + nb],
                                            INV / 784.0, None, op0=ALU.mult)
                    y1s_mm(cc, b0, nb, start=False, stop=True)
            if cc < 3:
                nc.vector.tensor_scalar(xmb[:, cc, :], xmsum[:, cc, :],
                                        INV / 784.0, None, op0=ALU.mult)
                y1s_mm(cc, 0, 16, start=(cc == 0), stop=False)

        y1s = ap_.tile([128, 128], bf16, tag="y1s_sb")
        nc.scalar.activation(y1s[:], y1s_ps[:], AF.Prelu, alpha=0.01)

        # ---------------- shallow chain tail ----------------
        y2s_ps = pp.tile([128, 64], f32, tag="mm")
        for m in range(4):
            for k in range(8):
                nc.tensor.matmul(y2s_ps[:, m * 16:(m + 1) * 16],
                                 sw2_t[:, k, m * 128:(m + 1) * 128],
                                 y1s[:, k * 16:(k + 1) * 16],
                                 start=(k == 0), stop=(k == 7))
        y2s = ap_.tile([128, 64], bf16, tag="y2s")
        nc.scalar.activation(y2s[:], y2s_ps[:], AF.Prelu, scale=INV, alpha=0.01)

        sh_ps = pt.tile([128, 16], f32, tag="tail")
        for k in range(4):
            nc.tensor.matmul(sh_ps[:64, :], sw3_t[:, k, :],
                             y2s[:, k * 16:(k + 1) * 16],
                             start=(k == 0), stop=(k == 3))
        shallow = ap_.tile([64, 16], f32, tag="shallow")
        nc.scalar.activation(shallow[:], sh_ps[:64, :], AF.Prelu,
                             scale=INV, alpha=0.01)

        # texture (folded tw1 + ACT bias)
        t1_ps = pt.tile([128, 16], f32, tag="tail")
        nc.tensor.matmul(t1_ps[:64, :], tw1_l, shallow[:], start=True, stop=True)
        t1 = ap_.tile([64, 16], f32, tag="t1")
        nc.scalar.activation(t1[:], t1_ps[:64, :], AF.Prelu, bias=tbias,
                             alpha=0.01)
        tex = ap_.tile([64, 16], f32, tag="tex")
        t2_ps = pt.tile([128, 16], f32, tag="tail")
        nc.tensor.matmul(t2_ps[:64, :], tw2_l, t1[:], start=True, stop=True)
        nc.scalar.activation(tex[:], t2_ps[:64, :], AF.Prelu, alpha=0.01)

        # sim'[b,k] = -2 t.p + |p|^2 (dropping |t|^2: argmax and
        # log-sum-exp(sim - max) are invariant to per-sample shifts)
        sim_ps = pt.tile([128, 16], f32, tag="tail")
        nc.tensor.matmul(sim_ps[0:16, 0:4], tex[:], rsim, start=True,
                         stop=False)
        nc.tensor.matmul(sim_ps[0:16, 0:4], onerow, pnorm, start=False,
                         stop=True)
        sim_sb = ap_.tile([16, 4], f32, tag="sim_sb")
        nc.vector.tensor_copy(sim_sb[:], sim_ps[0:16, 0:4])

        m16 = ap_.tile([16, 1], f32, tag="m16")
        nc.vector.reduce_max(m16[:], sim_sb[:], axis=AX.X)
        negm = ap_.tile([16, 1], f32, tag="negm")
        nc.vector.reduce_max(negm[:], sim_sb[:], axis=AX.X, negate=True)
        onehotT = ap_.tile([16, 4], f32, tag="onehotT")
        nc.vector.tensor_scalar(onehotT[:], sim_sb[:], m16[:, 0:1], None,
                                op0=ALU.is_ge)
        oh_ps = pt.tile([128, 16], f32, tag="tail")
        nc.tensor.transpose(oh_ps[0:4, 0:16], onehotT[:], id16)
        oh_sb = ap_.tile([4, 16], f32, tag="oh_sb")
        nc.vector.tensor_copy(oh_sb[:], oh_ps[0:4, 0:16])

        # finish class chain: bias select via one-hot, then cw2 + svdd
        nc.tensor.matmul(cat2_ps[:], cbias, oh_sb[:], start=False, stop=True)
        c1 = ap_.tile([64, 16], f32, tag="c1")
        nc.scalar.activation(c1[:], cat2_ps[:], AF.Prelu, alpha=0.01)
        cw2_ps = pt.tile([128, 16], f32, tag="tail")
        nc.tensor.matmul(cw2_ps[:64, :], cw2_l, c1[:], start=True, stop=True)
        cf = ap_.tile([64, 16], f32, tag="cf")
        nc.scalar.activation(cf[:], cw2_ps[:64, :], AF.Prelu, alpha=0.01)
        csq = ap_.tile([64, 16], f32, tag="csq")
        nc.scalar.activation(csq[:], cf[:], AF.Square, bias=ncen)
        csv_ps = pt.tile([128, 16], f32, tag="tail")
        nc.tensor.matmul(csv_ps[0:1, :], ones64, csq[:], start=True, stop=True)

        # CE on ACT (same table set; runs parallel to class chain)
        e_t = ap_.tile([16, 4], f32, tag="e_t")
        s16 = ap_.tile([16, 1], f32, tag="s16")
        nc.scalar.activation(e_t[:], sim_sb[:], AF.Exp, bias=negm[:, 0:1],
                             accum_out=s16[:])
        ce_col = ap_.tile([16, 1], f32, tag="ce_col")
        nc.scalar.activation(ce_col[:], s16[:], AF.Ln)
        ce_ps = pt.tile([128, 16], f32, tag="tail")
        nc.tensor.matmul(ce_ps[0:1, 0:1], ce_col[:], ones16, start=True,
                         stop=True)

        # align + partial sums (read csvdd straight from PSUM)
        al = ap_.tile([1, 16], f32, tag="al")
        nc.vector.tensor_tensor(al[:], osvdd[:], csv_ps[0:1, :], op=ALU.subtract)
        al2 = ap_.tile([1, 16], f32, tag="al2")
        nc.scalar.activation(al2[:], al[:], AF.Abs)

        outv = ap_.tile([1, 4], f32, tag="outv")
        nc.vector.reduce_sum(outv[0:1, 1:2], osvdd[:], axis=AX.X)
        nc.vector.reduce_sum(outv[0:1, 2:3], csv_ps[0:1, :], axis=AX.X)
        nc.vector.reduce_sum(outv[0:1, 3:4], al2[:], axis=AX.X)
        nc.vector.tensor_copy(outv[0:1, 0:1], ce_ps[0:1, 0:1])
        nc.sync.dma_start(out=out_d[:], in_=outv[:])

    nc.compile()
    return nc


def _host_prep(inputs):
    f = np.float32
    xm = np.ascontiguousarray(np.asarray(inputs["x_mid"], f).reshape(B, 512, 784))
    xd = np.ascontiguousarray(np.asarray(inputs["x_deep"], f).reshape(B, 100352))

    def T8(w, kparts, out):
        # w: [out, in] -> lhsT [in, out] -> [128, kparts, out], f = 128k + p
        wT = np.asarray(w, f).T
        arr = wT.reshape(kparts, 128, out).transpose(1, 0, 2)
        return np.ascontiguousarray((arr * WSCALE).astype(F8))

    # ow1: d = 16p + j (matches the x_deep DMA layout)
    ow1T = np.asarray(inputs["ow1"], f).T.reshape(128, 16, 1024)
    ow1T = np.ascontiguousarray((ow1T * WSCALE).astype(F8))
    # folded first shallow layer: Wf = sw1 @ w_shallow, c = 128cc + p
    Wf = (np.asarray(inputs["sw1"], f) @ np.asarray(inputs["w_shallow"], f))
    wfT = T8(Wf, 4, 1024)

    center = np.asarray(inputs["center"], f)
    proto = np.asarray(inputs["proto"], f)
    tw1 = np.asarray(inputs["tw1"], f)
    cw1 = np.asarray(inputs["cw1"], f)
    tw1a, tw1b = tw1[:, :64], tw1[:, 64:]
    cw1a, cw1b = cw1[:, :64], cw1[:, 64:]

    cst = np.zeros((128, C_K), f)
    cst[0:64, C_TW1:C_TW1 + 64] = (tw1a + tw1b).T
    cst[0:64, C_TW2:C_TW2 + 64] = np.asarray(inputs["tw2"], f).T
    cst[0:64, C_CW1:C_CW1 + 64] = (cw1a + cw1b).T
    cst[0:64, C_CW2:C_CW2 + 64] = np.asarray(inputs["cw2"], f).T
    cst[0:64, C_QW1:C_QW1 + 64] = np.asarray(inputs["qw1"], f).T
    cst[0:64, C_QW2:C_QW2 + 64] = np.asarray(inputs["qw2"], f).T
    cst[0:64, C_RSIM:C_RSIM + 4] = -2.0 * proto.T
    cst[64, C_RSIM:C_RSIM + 4] = 1.0
    cst[0:4, C_CBIAS:C_CBIAS + 64] = -(proto @ cw1b.T)
    cst[0:64, C_TBIAS] = -(tw1b @ center)
    cst[0:64, C_CEN] = center
    cst[0:64, C_NCEN] = -center
    cst[0, C_PNORM:C_PNORM + 4] = (proto ** 2).sum(axis=1)
    cst[0:16, C_ID16:C_ID16 + 16] = np.eye(16, dtype=f)
    cst[0:64, C_ONES] = 1.0
    cst[0, C_ONEROW:C_ONEROW + 16] = 1.0

    shared = {
        "ow1T": ow1T,
        "wfT": wfT,
        "ow2T": T8(inputs["ow2"], 8, 512),
        "sw2T": T8(inputs["sw2"], 8, 512),
        "ow3T": T8(inputs["ow3"], 4, 64),
        "sw3T": T8(inputs["sw3"], 4, 64),
        "cst": cst,
    }
    in_maps = []
    for c in range(N_CORES):
        m = dict(shared)
        m["xm"] = np.ascontiguousarray(xm[c * BC:(c + 1) * BC])
        m["xd"] = np.ascontiguousarray(xd[c * BC:(c + 1) * BC])
        in_maps.append(m)
    return in_maps


def _get_program():
    if "nc" not in _CACHE:
        _CACHE["nc"] = _build_program()
    return _CACHE["nc"]


def _combine(parts):
    tot = np.sum([np.asarray(p, np.float64).ravel() for p in parts], axis=0)
    return (tot / B).astype(np.float32).reshape(4, 1)


def _run(inputs, trace=False):
    from concourse.bass_utils import run_bass_kernel_spmd
    nc = _get_program()
    in_maps = _host_prep(inputs)
    kw = {}
    if trace:
        kw = dict(trace=True, trace_cores=list(range(N_CORES)))
    res = run_bass_kernel_spmd(nc, in_maps, list(range(N_CORES)), **kw)
    out = _combine([res.results[i]["out"] for i in range(N_CORES)])
    return out, res


def kernel(**inputs):
    out, _ = _run(inputs, trace=False)
    return out


def kernel_traced(**inputs):
    """Returns (output, exec_time_ns) using the NTFF profile (max over cores)."""
    out, res = _run(inputs, trace=True)
    return out, res.exec_time_ns
